# revision 37
# baseline (speedup 1.0000x reference)
"""Trainium2 Bass kernel for the bilinear/demosaic stencil problem.

Full inputs: mosic [16,3,1024,1024] f32, mask [16,3,1024,1024] f32.
Output: clip(mosic + interp*(1-mask), 0, 255)/255, where interp is
  g = g0 + convG(g0)
  r = t + convG(t), t = r0 + convRB(r0)   (same for b)
with convG = cross 3x3 /4, convRB = diagonal 3x3 /4, zero padding.

Sharding: pure data parallel — 2 batch images per core across 8 cores.

Per-core algorithm: the pre-blend value v = mosic + interp is linear in the
input plane X, and expands (exactly) over horizontal shifts as

  v_g  = (2I + 0.25V) X  + 0.25 X<L>  + 0.25 X<R>
  v_rb = (2I + 0.375V) X + A X<L> + A X<R> + 0.0625V X<LL> + 0.0625V X<RR>
         - 0.0625 V X restricted to columns 0 and W-1
  with A = 0.25I + 0.25V + 0.0625V^2

where V is the tridiagonal vertical-neighbor band matrix and <L>/<R> are
column shifts (zero padded).  Every term is one TensorE matmul (band matrix
stationary, column-shifted tile as the moving operand) accumulated in PSUM,
so the whole stencil runs on the PE in float32r at 1 cycle/row (the L+R
pair additionally pre-summed by one DVE tensor_tensor into H1).  The final
blend is out = mask ? mosic/255 : min(v/255, 1)  (the lower clip is a no-op
because every term is nonnegative): copy_predicated overwrites masked
pixels with exact fp32 mosic directly in PSUM, then the otherwise-idle ACT
engine computes min(v,255)/255 per psum bank as 1 - Relu(255 - v)/255.

DMA routing (measured fastest): mosic loads on the sync HWDGE ring, mask
loads on the scalar HWDGE ring (both pure prefetch, issued 2 chunks ahead),
output stores on the gpsimd static queue, deferred one chunk and split into
sub-DMAs so their semaphore waits are pre-satisfied at ring arrival.

Images are processed in vertical chunks of 128 input rows with 2-row overlap
(the V-band matmul is only valid away from tile edges); chunk 0 and the last
chunk use the true image boundary, which the finite band matrices handle
exactly.
"""

import numpy as np

import concourse.bass as bass
import concourse.bacc as bacc
import concourse.mybir as mybir
import concourse.tile as tile
from concourse.bass_utils import run_bass_kernel_spmd

F32 = mybir.dt.float32

B, C, H, W = 16, 3, 1024, 1024
N_CORES = 8
BPC = B // N_CORES  # images per core

# matrix slots in the packed weight tensor
G0, GL, A0, AL, AV2, AVC = range(6)

# horizontal pad columns on each side of a channel block
PAD = 2
WB = W + 2 * PAD  # channel block width in the X tile


def _wmats(P: int) -> np.ndarray:
    """Packed [P, 6*P] stationary matrices (all symmetric, so lhsT == M)."""
    I = np.eye(P, dtype=np.float64)
    V = np.zeros((P, P), np.float64)
    idx = np.arange(P - 1)
    V[idx, idx + 1] = 1.0
    V[idx + 1, idx] = 1.0
    V2 = V @ V
    mats = [
        2 * I + 0.25 * V,                    # G0
        0.25 * I,                            # GL
        2 * I + 0.375 * V,                   # A0
        0.25 * I + 0.25 * V + 0.0625 * V2,   # AL
        0.0625 * V,                          # AV2
        -0.0625 * V,                         # AVC (edge-column correction)
    ]
    return np.concatenate(mats, axis=1).astype(np.float32)


def _chunks():
    """(in_row_start a, in_rows P, out_row_start o, out_rows OR, valid_off vo)."""
    out = [(0, 128, 0, 126, 0)]
    o = 126
    while o + 124 <= H - 30:
        out.append((o - 2, 128, o, 124, 2))
        o += 124
    a = H - 32
    out.append((a, 32, o, H - o, o - a))
    return out


def _build_nc(mm_dt=mybir.dt.float32r):
    nc = bacc.Bacc(trn_type="TRN2")
    # mosic/weights are typed as the matmul dtype (float32r = fp32 storage,
    # PE rounds to 11 mantissa bits on read); host arrays stay np.float32.
    mos = nc.dram_tensor("mosic", [BPC, C, H, W], mm_dt, kind="ExternalInput")
    msk = nc.dram_tensor("mask", [BPC, C, H, W], F32, kind="ExternalInput")
    w128 = nc.dram_tensor("w128", [128, 6 * 128], mm_dt, kind="ExternalInput")
    w32 = nc.dram_tensor("w32", [32, 6 * 32], mm_dt, kind="ExternalInput")
    out = nc.dram_tensor("out", [BPC, C, H, W], F32, kind="ExternalOutput")

    inv255 = 1.0 / 255.0

    with tile.TileContext(nc) as tc:
        with (
            tc.tile_pool(name="wp", bufs=1) as wp,
            tc.tile_pool(name="xp", bufs=4) as xp,
            tc.tile_pool(name="mp", bufs=4) as mp,
            tc.tile_pool(name="op", bufs=3) as op_,
            tc.tile_pool(name="h1p", bufs=2) as h1p,
            tc.tile_pool(name="psp", bufs=8, space="PSUM") as psp,
        ):
            wt128 = wp.tile([128, 6 * 128], mm_dt)
            wt32 = wp.tile([32, 6 * 32], mm_dt)
            b255 = wp.tile([128, 1], F32)
            nc.gpsimd.memset(b255[:], 255.0)

            chunks_all = [(img, ch) for img in range(BPC) for ch in _chunks()]
            NCH = len(chunks_all)
            PF = 2  # load prefetch depth (chunks)

            def load_X(k):
                img, (a, P, o, OR, vo) = chunks_all[k]
                X = xp.tile([128, C, WB], mm_dt, tag="X", name=f"X{k}")
                nc.gpsimd.memset(X[0:P, :, 0:PAD].bitcast(F32), 0.0)
                nc.gpsimd.memset(X[0:P, :, PAD + W:WB].bitcast(F32), 0.0)
                nc.sync.dma_start(
                    X[0:P, :, PAD:PAD + W],
                    mos[img][:, a:a + P, :].rearrange("c p w -> p c w"),
                )
                return X

            def load_M(k):
                img, (a, P, o, OR, vo) = chunks_all[k]
                M = mp.tile([128, C, W], F32, tag="M", name=f"M{k}")
                # Load the chunk's full input-row range at partition 0: DMAs
                # with a nonzero partition base take a much slower path, and
                # rows a..a+P align M with the psum partitions directly.
                nc.scalar.dma_start(
                    M[0:P],
                    msk[img][:, a:a + P, :].rearrange("c p w -> p c w"),
                )
                return M

            # X(0) goes first on the sync ring; weights ride the scalar
            # ring ahead of the masks so the first matmul waits on neither.
            xtiles = {0: load_X(0)}
            nc.scalar.dma_start(wt128[:], w128[:])
            nc.scalar.dma_start(wt32[:], w32[:])
            mtiles = {0: load_M(0)}
            for k in range(1, PF):
                xtiles[k] = load_X(k)
                mtiles[k] = load_M(k)

            pending_store = []

            def flush_store(keep=0):
                while len(pending_store) > keep:
                    Os, simg, so, sOR, svo = pending_store.pop(0)
                    # Split at 32-aligned SBUF partitions: partition-base-
                    # aligned sub-DMAs take the fast descriptor path.
                    cuts = [svo] + [p for p in (32, 64, 96) if svo < p < svo + sOR] \
                        + [svo + sOR]
                    for sv, sv1 in zip(cuts, cuts[1:]):
                        r0 = so + (sv - svo)
                        nc.gpsimd.dma_start(
                            out[simg][:, r0:r0 + (sv1 - sv), :].rearrange("c p w -> p c w"),
                            Os[sv:sv1],
                        )

            for ci in range(NCH):
                img, (a, P, o, OR, vo) = chunks_all[ci]
                flush_store(keep=0)
                if ci + PF < NCH:
                    xtiles[ci + PF] = load_X(ci + PF)
                    mtiles[ci + PF] = load_M(ci + PF)
                X = xtiles.pop(ci)
                M = mtiles.pop(ci)
                wt = wt128 if P == 128 else wt32

                def lhs(k):
                    return wt[0:P, k * P:(k + 1) * P]

                if True:
                    Xf = X[0:P].rearrange("p c w -> p (c w)")
                    XfF = Xf.bitcast(F32)
                    H1 = h1p.tile([128, C * WB - 2], mm_dt, tag="H1")
                    nc.vector.tensor_tensor(
                        H1[0:P], XfF[:, 0:C * WB - 2], XfF[:, 2:C * WB],
                        mybir.AluOpType.add,
                    )
                    ps = []
                    for c in range(C):
                        cb = c * WB
                        if c == 1:
                            # (matrix, tensor, flat col of first output col)
                            terms = [(G0, Xf, cb + PAD), (GL, H1, cb + 1)]
                        else:
                            terms = [(A0, Xf, cb + PAD), (AL, H1, cb + 1),
                                     (AV2, Xf, cb + PAD + 2), (AV2, Xf, cb + PAD - 2)]
                        half = []
                        for h in range(2):
                            n0 = h * 512
                            p = psp.tile([128, 512], F32, tag="ps")
                            half.append(p)
                            corr = c != 1
                            for i, (k, src_t, fo) in enumerate(terms):
                                if src_t is Xf:
                                    rhs = Xf[:, fo + n0:fo + n0 + 512]
                                else:
                                    rhs = src_t[0:P, fo + n0:fo + n0 + 512]
                                nc.tensor.matmul(
                                    p[0:P, :],
                                    lhs(k),
                                    rhs,
                                    start=(i == 0),
                                    stop=(i == len(terms) - 1 and not corr),
                                )
                            if corr:
                                # LR/RL expansion over-counts V at the image's
                                # first/last column; subtract 0.0625*V there.
                                # N=1 violates fp32r moving-dim restrictions;
                                # use a plain fp32 matmul (exact) instead.
                                ecol = PAD if h == 0 else PAD + W - 1
                                ocol = 0 if h == 0 else 511
                                nc.tensor.matmul(
                                    p[0:P, ocol:ocol + 1],
                                    lhs(AVC).bitcast(F32),
                                    X[0:P, c, ecol:ecol + 1].bitcast(F32),
                                    start=False,
                                    stop=True,
                                )
                        ps.append(half)

                    # Blend: overwrite masked pixels with exact mosic directly
                    # in PSUM (mask is exactly 0.0/1.0 -> int32 bitcast keeps
                    # truthiness), then clip+scale each half in one DVE op.
                    O = op_.tile([128, C, W], F32, tag="O")
                    for c in range(C):
                        for h in range(2):
                            n0 = h * 512
                            nc.vector.copy_predicated(
                                ps[c][h][0:P, :],
                                M[0:P, c, n0:n0 + 512].bitcast(mybir.dt.int32),
                                X[0:P, c, PAD + n0:PAD + n0 + 512].bitcast(F32),
                            )
                            # min(v,255)/255 on the idle ACT engine:
                            #   z = Relu(255 - v);  out = 1 - z/255
                            nc.scalar.activation(
                                O[0:P, c, n0:n0 + 512], ps[c][h][0:P, :],
                                mybir.ActivationFunctionType.Relu,
                                bias=b255[0:P, 0:1], scale=-1.0,
                            )
                            nc.scalar.activation(
                                O[0:P, c, n0:n0 + 512], O[0:P, c, n0:n0 + 512],
                                mybir.ActivationFunctionType.Copy,
                                bias=1.0, scale=-inv255,
                            )
                    # Defer the store by one chunk (so its wait is already
                    # satisfied at ring arrival) and split it into sub-DMAs
                    # (more packets -> more SDMA engines on the static queue).
                    pending_store.append((O, img, o, OR, vo))

            flush_store()

    nc.finalize()
    return nc


_CACHE: dict = {}


def _get_nc(mm_dt=mybir.dt.float32r):
    key = str(mm_dt)
    if key not in _CACHE:
        _CACHE[key] = _build_nc(mm_dt)
    return _CACHE[key]


def _run(mosic, mask, mm_dt=mybir.dt.float32r, **spmd_kwargs):
    nc = _get_nc(mm_dt)
    mosic = np.ascontiguousarray(np.asarray(mosic, dtype=np.float32))
    mask = np.ascontiguousarray(np.asarray(mask, dtype=np.float32))
    w128 = _wmats(128)
    w32 = _wmats(32)
    in_maps = []
    for cid in range(N_CORES):
        sl = slice(cid * BPC, (cid + 1) * BPC)
        in_maps.append({
            "mosic": mosic[sl],
            "mask": mask[sl],
            "w128": w128,
            "w32": w32,
        })
    res = run_bass_kernel_spmd(nc, in_maps, core_ids=list(range(N_CORES)), **spmd_kwargs)
    full = np.concatenate([r["out"] for r in res.results], axis=0)
    return full, res


def kernel(mosic, mask):
    full, _ = _run(mosic, mask)
    return full


# revision 38
# speedup vs baseline: 1.1156x; 1.1156x over previous
"""Trainium2 Bass kernel for the bilinear/demosaic stencil problem.

Full inputs: mosic [16,3,1024,1024] f32, mask [16,3,1024,1024] f32.
Output: clip(mosic + interp*(1-mask), 0, 255)/255, where interp is
  g = g0 + convG(g0)
  r = t + convG(t), t = r0 + convRB(r0)   (same for b)
with convG = cross 3x3 /4, convRB = diagonal 3x3 /4, zero padding.

Sharding: pure data parallel — 2 batch images per core across 8 cores.

Per-core algorithm: the pre-blend value v = mosic + interp is linear in the
input plane X, and expands (exactly) over horizontal shifts as

  v_g  = (2I + 0.25V) X  + 0.25 X<L>  + 0.25 X<R>
  v_rb = (2I + 0.375V) X + A X<L> + A X<R> + 0.0625V X<LL> + 0.0625V X<RR>
         - 0.0625 V X restricted to columns 0 and W-1
  with A = 0.25I + 0.25V + 0.0625V^2

where V is the tridiagonal vertical-neighbor band matrix and <L>/<R> are
column shifts (zero padded).  Every term is one TensorE matmul (band matrix
stationary, column-shifted tile as the moving operand) accumulated in PSUM,
so the whole stencil runs on the PE in float32r at 1 cycle/row (the L+R
pair additionally pre-summed by one DVE tensor_tensor into H1).  The final
blend is out = mask ? mosic/255 : min(v/255, 1)  (the lower clip is a no-op
because every term is nonnegative): copy_predicated overwrites masked
pixels with exact fp32 mosic directly in PSUM, then the otherwise-idle ACT
engine computes min(v,255)/255 per psum bank as 1 - Relu(255 - v)/255.

DMA routing (measured fastest): mosic loads on the sync HWDGE ring, mask
loads on the scalar HWDGE ring (both pure prefetch, issued 2 chunks ahead),
output stores on the gpsimd static queue, deferred one chunk and split into
sub-DMAs so their semaphore waits are pre-satisfied at ring arrival.

Images are processed in vertical chunks of 128 input rows with 2-row overlap
(the V-band matmul is only valid away from tile edges); chunk 0 and the last
chunk use the true image boundary, which the finite band matrices handle
exactly.
"""

import numpy as np

import concourse.bass as bass
import concourse.bacc as bacc
import concourse.mybir as mybir
import concourse.tile as tile
from concourse.bass_utils import run_bass_kernel_spmd

F32 = mybir.dt.float32

B, C, H, W = 16, 3, 1024, 1024
N_CORES = 8
BPC = B // N_CORES  # images per core

# matrix slots in the packed weight tensor
G0, GL, A0, AL, AV2, AVC = range(6)

# horizontal pad columns on each side of a channel block
PAD = 2
WB = W + 2 * PAD  # channel block width in the X tile


def _wmats(P: int) -> np.ndarray:
    """Packed [P, 6*P] stationary matrices (all symmetric, so lhsT == M)."""
    I = np.eye(P, dtype=np.float64)
    V = np.zeros((P, P), np.float64)
    idx = np.arange(P - 1)
    V[idx, idx + 1] = 1.0
    V[idx + 1, idx] = 1.0
    V2 = V @ V
    mats = [
        2 * I + 0.25 * V,                    # G0
        0.25 * I,                            # GL
        2 * I + 0.375 * V,                   # A0
        0.25 * I + 0.25 * V + 0.0625 * V2,   # AL
        0.0625 * V,                          # AV2
        -0.0625 * V,                         # AVC (edge-column correction)
    ]
    return np.concatenate(mats, axis=1).astype(np.float32)


def _chunks():
    """(in_row_start a, in_rows P, out_row_start o, out_rows OR, valid_off vo)."""
    out = [(0, 128, 0, 126, 0)]
    o = 126
    while o + 124 <= H - 30:
        out.append((o - 2, 128, o, 124, 2))
        o += 124
    a = H - 32
    out.append((a, 32, o, H - o, o - a))
    return out


def _build_nc(mm_dt=mybir.dt.float32r):
    nc = bacc.Bacc(trn_type="TRN2")
    # mosic/weights are typed as the matmul dtype (float32r = fp32 storage,
    # PE rounds to 11 mantissa bits on read); host arrays stay np.float32.
    mos = nc.dram_tensor("mosic", [BPC, C, H, W], mm_dt, kind="ExternalInput")
    msk = nc.dram_tensor("mask", [BPC, C, H, W], F32, kind="ExternalInput")
    w128 = nc.dram_tensor("w128", [128, 6 * 128], mm_dt, kind="ExternalInput")
    w32 = nc.dram_tensor("w32", [32, 6 * 32], mm_dt, kind="ExternalInput")
    out = nc.dram_tensor("out", [BPC, C, H, W], F32, kind="ExternalOutput")

    inv255 = 1.0 / 255.0

    with tile.TileContext(nc) as tc:
        with (
            tc.tile_pool(name="wp", bufs=1) as wp,
            tc.tile_pool(name="xp", bufs=4) as xp,
            tc.tile_pool(name="mp", bufs=4) as mp,
            tc.tile_pool(name="op", bufs=3) as op_,
            tc.tile_pool(name="h1p", bufs=2) as h1p,
            tc.tile_pool(name="psp", bufs=8, space="PSUM") as psp,
        ):
            wt128 = wp.tile([128, 6 * 128], mm_dt)
            nc.sync.dma_start(wt128[:], w128[:])
            wt32 = wp.tile([32, 6 * 32], mm_dt)
            nc.sync.dma_start(wt32[:], w32[:])
            b255 = wp.tile([128, 1], F32)
            nc.gpsimd.memset(b255[:], 255.0)

            chunks_all = [(img, ch) for img in range(BPC) for ch in _chunks()]
            NCH = len(chunks_all)
            PF = 2  # load prefetch depth (chunks)

            def load_X(k):
                img, (a, P, o, OR, vo) = chunks_all[k]
                X = xp.tile([128, C, WB], mm_dt, tag="X", name=f"X{k}")
                nc.gpsimd.memset(X[0:P, :, 0:PAD].bitcast(F32), 0.0)
                nc.gpsimd.memset(X[0:P, :, PAD + W:WB].bitcast(F32), 0.0)
                nc.sync.dma_start(
                    X[0:P, :, PAD:PAD + W],
                    mos[img][:, a:a + P, :].rearrange("c p w -> p c w"),
                )
                return X

            def load_M(k):
                img, (a, P, o, OR, vo) = chunks_all[k]
                M = mp.tile([128, C, W], F32, tag="M", name=f"M{k}")
                # Load the chunk's full input-row range at partition 0: DMAs
                # with a nonzero partition base take a much slower path, and
                # rows a..a+P align M with the psum partitions directly.
                nc.scalar.dma_start(
                    M[0:P],
                    msk[img][:, a:a + P, :].rearrange("c p w -> p c w"),
                )
                return M

            xtiles = {k: load_X(k) for k in range(PF)}
            mtiles = {k: load_M(k) for k in range(PF)}

            pending_store = []

            def flush_store(keep=0):
                while len(pending_store) > keep:
                    Os, simg, so, sOR, svo = pending_store.pop(0)
                    # Split at 32-aligned SBUF partitions: partition-base-
                    # aligned sub-DMAs take the fast descriptor path.
                    cuts = [svo] + [p for p in (32, 64, 96) if svo < p < svo + sOR] \
                        + [svo + sOR]
                    for sv, sv1 in zip(cuts, cuts[1:]):
                        r0 = so + (sv - svo)
                        nc.gpsimd.dma_start(
                            out[simg][:, r0:r0 + (sv1 - sv), :].rearrange("c p w -> p c w"),
                            Os[sv:sv1],
                        )

            for ci in range(NCH):
                img, (a, P, o, OR, vo) = chunks_all[ci]
                flush_store(keep=0)
                if ci + PF < NCH:
                    xtiles[ci + PF] = load_X(ci + PF)
                    mtiles[ci + PF] = load_M(ci + PF)
                X = xtiles.pop(ci)
                M = mtiles.pop(ci)
                wt = wt128 if P == 128 else wt32

                def lhs(k):
                    return wt[0:P, k * P:(k + 1) * P]

                if True:
                    Xf = X[0:P].rearrange("p c w -> p (c w)")
                    XfF = Xf.bitcast(F32)
                    H1 = h1p.tile([128, C * WB - 2], mm_dt, tag="H1")
                    nc.vector.tensor_tensor(
                        H1[0:P], XfF[:, 0:C * WB - 2], XfF[:, 2:C * WB],
                        mybir.AluOpType.add,
                    )
                    ps = []
                    for c in range(C):
                        cb = c * WB
                        if c == 1:
                            # (matrix, tensor, flat col of first output col)
                            terms = [(G0, Xf, cb + PAD), (GL, H1, cb + 1)]
                        else:
                            terms = [(A0, Xf, cb + PAD), (AL, H1, cb + 1),
                                     (AV2, Xf, cb + PAD + 2), (AV2, Xf, cb + PAD - 2)]
                        half = []
                        for h in range(2):
                            n0 = h * 512
                            p = psp.tile([128, 512], F32, tag="ps")
                            half.append(p)
                            corr = c != 1
                            for i, (k, src_t, fo) in enumerate(terms):
                                if src_t is Xf:
                                    rhs = Xf[:, fo + n0:fo + n0 + 512]
                                else:
                                    rhs = src_t[0:P, fo + n0:fo + n0 + 512]
                                nc.tensor.matmul(
                                    p[0:P, :],
                                    lhs(k),
                                    rhs,
                                    start=(i == 0),
                                    stop=(i == len(terms) - 1 and not corr),
                                )
                            if corr:
                                # LR/RL expansion over-counts V at the image's
                                # first/last column; subtract 0.0625*V there.
                                # N=1 violates fp32r moving-dim restrictions;
                                # use a plain fp32 matmul (exact) instead.
                                ecol = PAD if h == 0 else PAD + W - 1
                                ocol = 0 if h == 0 else 511
                                nc.tensor.matmul(
                                    p[0:P, ocol:ocol + 1],
                                    lhs(AVC).bitcast(F32),
                                    X[0:P, c, ecol:ecol + 1].bitcast(F32),
                                    start=False,
                                    stop=True,
                                )
                        ps.append(half)

                    # Blend: overwrite masked pixels with exact mosic directly
                    # in PSUM (mask is exactly 0.0/1.0 -> int32 bitcast keeps
                    # truthiness), then clip+scale each half in one DVE op.
                    O = op_.tile([128, C, W], F32, tag="O")
                    for c in range(C):
                        for h in range(2):
                            n0 = h * 512
                            nc.vector.copy_predicated(
                                ps[c][h][0:P, :],
                                M[0:P, c, n0:n0 + 512].bitcast(mybir.dt.int32),
                                X[0:P, c, PAD + n0:PAD + n0 + 512].bitcast(F32),
                            )
                            # min(v,255)/255 on the idle ACT engine:
                            #   z = Relu(255 - v);  out = 1 - z/255
                            nc.scalar.activation(
                                O[0:P, c, n0:n0 + 512], ps[c][h][0:P, :],
                                mybir.ActivationFunctionType.Relu,
                                bias=b255[0:P, 0:1], scale=-1.0,
                            )
                            nc.scalar.activation(
                                O[0:P, c, n0:n0 + 512], O[0:P, c, n0:n0 + 512],
                                mybir.ActivationFunctionType.Copy,
                                bias=1.0, scale=-inv255,
                            )
                    # Defer the store by one chunk (so its wait is already
                    # satisfied at ring arrival) and split it into sub-DMAs
                    # (more packets -> more SDMA engines on the static queue).
                    pending_store.append((O, img, o, OR, vo))

            flush_store()

    nc.finalize()
    return nc


_CACHE: dict = {}


def _get_nc(mm_dt=mybir.dt.float32r):
    key = str(mm_dt)
    if key not in _CACHE:
        _CACHE[key] = _build_nc(mm_dt)
    return _CACHE[key]


def _run(mosic, mask, mm_dt=mybir.dt.float32r, **spmd_kwargs):
    nc = _get_nc(mm_dt)
    mosic = np.ascontiguousarray(np.asarray(mosic, dtype=np.float32))
    mask = np.ascontiguousarray(np.asarray(mask, dtype=np.float32))
    w128 = _wmats(128)
    w32 = _wmats(32)
    in_maps = []
    for cid in range(N_CORES):
        sl = slice(cid * BPC, (cid + 1) * BPC)
        in_maps.append({
            "mosic": mosic[sl],
            "mask": mask[sl],
            "w128": w128,
            "w32": w32,
        })
    res = run_bass_kernel_spmd(nc, in_maps, core_ids=list(range(N_CORES)), **spmd_kwargs)
    full = np.concatenate([r["out"] for r in res.results], axis=0)
    return full, res


def kernel(mosic, mask):
    full, _ = _run(mosic, mask)
    return full
